# revision 1
# baseline (speedup 1.0000x reference)
"""Trainium2 Bass kernel for CrossAttention (B=8, N=M=2048, C=1024), fp32.

Sharding: data-parallel — one batch element per NeuronCore (8 cores).

Per-core computation (batch b):
  q  = x[b] @ wq^T          -> held transposed:  qT[d, n]
  kT[d, m] = (support[b] @ wk^T)^T
  v[m, d]  = (support[b] @ wv^T) * mask[m]   (post-softmax column mask == row
                                              mask on v; softmax denominator
                                              still spans all m)
  p  = exp(SCALE * qT^T kT)  (no max subtraction: logits ~ N(0, 8), safe fp32)
  o[n, d] = (p @ v) / rowsum(p)
  The reference's  out.swapaxes(1,2).reshape(N, C)  permutation satisfies
  o_perm[2t+i, c] = o[1024*i + c, t], so the final projection becomes
  final[2t+i, d'] = sum_c o[1024*i + c, t] * proj_w[d', c]  — a plain matmul
  with o-block-i rows as the contraction dim, written out with row stride 2.

Matmul operands are float32r (full-rate PE streaming; plain fp32 is 4 cyc/row).
Host-side prep transposes x/support/weights once (fp32 has no DMA-transpose on
TRN2) and lays weights out in consumption order so chunked DMAs pipeline with
the first accumulation groups at phase boundaries.
"""

import sys

sys.path.insert(0, "/opt/trn_rl_repo")

import numpy as np

import concourse.bass as bass
import concourse.tile as tile
from concourse import bacc, mybir
from concourse.bass_utils import run_bass_kernel_spmd
from concourse.masks import make_identity

F32 = mybir.dt.float32
F32R = mybir.dt.float32r
AF = mybir.ActivationFunctionType

B, N, M, C = 8, 2048, 2048, 1024
CT = C // 128          # 8 c-tiles (contraction / channel partition tiles)
MT = M // 128          # 16 m-tiles
SCALE = (C // 8) ** -0.5
NCHUNK = 256           # q rows computed per chunk in the attention phase
MS = 512               # m-chunk for kv build and the s matmul free dim
WCH = 16               # weight DMA chunks (arrival order == consumption order)

_CACHE = {}


def _build_program():
    nc = bacc.Bacc(
        "TRN2",
        target_bir_lowering=False,
        debug=False,
        enable_asserts=False,
        num_devices=8,
    )

    xT = nc.dram_tensor("xT", [128, CT, N], F32, kind="ExternalInput")
    sT = nc.dram_tensor("sT", [128, CT, M], F32, kind="ExternalInput")
    wqT = nc.dram_tensor("wqT", [128, CT * C], F32, kind="ExternalInput")
    wkT = nc.dram_tensor("wkT", [128, CT * C], F32, kind="ExternalInput")
    wvT = nc.dram_tensor("wvT", [128, CT * C], F32, kind="ExternalInput")
    pwT = nc.dram_tensor("pwT", [128, CT * C], F32, kind="ExternalInput")
    maskf = nc.dram_tensor("maskf", [128, MT], F32, kind="ExternalInput")
    biasb = nc.dram_tensor("biasb", [128, C], F32, kind="ExternalInput")
    out = nc.dram_tensor("out", [N, C], F32, kind="ExternalOutput")

    with tile.TileContext(nc, pool_alloc_mode="queue") as tc:
        _trace_kernel(tc, xT, sT, wqT, wkT, wvT, pwT, maskf, biasb, out)
    nc.compile()
    return nc


def _dma_w(nc, wtile, wdram, ch0=0, ch1=WCH):
    # chunked weight load: pipelines with the first consuming matmul groups
    cw = (CT * C) // WCH
    for ch in range(ch0, ch1):
        nc.sync.dma_start(
            wtile[:, ch * cw:(ch + 1) * cw],
            wdram[:, ch * cw:(ch + 1) * cw].bitcast(F32R),
        )


def _dma_act(nc, atile, adram, col0, cols):
    # per-ct chunked activation load (arrival order == psum-group order)
    for ct in range(CT):
        nc.sync.dma_start(
            atile[:, ct, :],
            adram[:, ct, col0:col0 + cols].bitcast(F32R),
        )


def _trace_kernel(tc, xT, sT, wqT, wkT, wvT, pwT, maskf, biasb, out):
    nc = tc.nc

    from contextlib import ExitStack

    with ExitStack() as ctx:
        persist = ctx.enter_context(tc.tile_pool(name="persist", bufs=1))

        ident0 = persist.tile([128, 128], F32, tag="ident0")
        make_identity(nc, ident0[:])
        ident = persist.tile([128, 128], F32R, tag="ident")
        nc.scalar.copy(ident[:], ident0[:])
        maskt = persist.tile([128, MT], F32, tag="maskt")
        nc.sync.dma_start(maskt[:], maskf[:])

        # o bounce buffer in DRAM (dependency-tracked tile)
        dram = ctx.enter_context(tc.tile_pool(name="dram", bufs=1, space="DRAM"))
        o_dram = dram.tile([N, C], F32, tag="o_dram")

        # v/kT live through attention; released before proj.  kT's pool opens
        # at phase K so phase V has room to preload wk alongside wv.
        kv_ctx = ctx.enter_context(ExitStack())
        vp = kv_ctx.enter_context(tc.tile_pool(name="vp", bufs=1))
        # v [m,d] as 16 col-blocks of [128, C]
        v = vp.tile([128, MT * C], F32R, tag="v")
        wk_ctx = ctx.enter_context(ExitStack())
        wkp = wk_ctx.enter_context(tc.tile_pool(name="wkp", bufs=1, side="right"))
        wk = wkp.tile([128, CT * C], F32R, tag="wk")

        # ---------------- phase V: v[m, d] = mask * (support @ wv^T) -------
        # wv is dc-major: [p, dc*4096 + ct*512 + dd]
        with (
            tc.tile_pool(name="wvp", bufs=1) as wvp,
            tc.tile_pool(name="stv", bufs=2) as stp,
            tc.tile_pool(name="vps", bufs=3, space="PSUM") as vps,
        ):
            wv = wvp.tile([128, CT * C], F32R, tag="wv")
            st0 = stp.tile([128, CT, MS], F32R, tag="st")
            cw = (CT * C) // WCH
            for i in range(CT):
                nc.sync.dma_start(
                    wv[:, i * cw:(i + 1) * cw],
                    wvT[:, i * cw:(i + 1) * cw].bitcast(F32R),
                )
                nc.sync.dma_start(
                    st0[:, i, :], sT[:, i, 0:MS].bitcast(F32R)
                )
            _dma_w(nc, wv, wvT, CT, WCH)
            for mc in range(M // MS):
                if mc == 0:
                    st = st0
                else:
                    st = stp.tile([128, CT, MS], F32R, tag="st")
                    _dma_act(nc, st, sT, mc * MS, MS)
                # spread next phase's weight prefetch across V's mc loop
                _dma_w(nc, wk, wkT, mc * 4, (mc + 1) * 4)
                for dc in range(C // 512):
                    for j in range(MS // 128):
                        mt = mc * (MS // 128) + j
                        ps = vps.tile([128, 512], F32, tag="vps")
                        for ct in range(CT):
                            nc.tensor.matmul(
                                ps[:],
                                lhsT=st[:, ct, j * 128:(j + 1) * 128],
                                rhs=wv[:, dc * 4096 + ct * 512: dc * 4096 + (ct + 1) * 512],
                                start=(ct == 0),
                                stop=(ct == CT - 1),
                            )
                        nc.vector.tensor_scalar_mul(
                            v[:, mt * C + dc * 512: mt * C + (dc + 1) * 512],
                            ps[:],
                            maskt[:, mt:mt + 1],
                        )

        # ---------------- phase K: kT[d, m] = (support @ wk^T)^T -----------
        # wk is dt-major: [p, dt*1024 + ct*128 + dd]; preloaded during V
        kTp = kv_ctx.enter_context(tc.tile_pool(name="kTp", bufs=1))
        # kT [d,m] as 8 col-blocks of [128, M]
        kT = kTp.tile([128, CT * M], F32R, tag="kT")
        with (
            tc.tile_pool(name="stk", bufs=2) as stp,
            tc.tile_pool(name="kps", bufs=3, space="PSUM") as kps,
        ):
            for mc in range(M // MS):
                st = stp.tile([128, CT, MS], F32R, tag="st")
                _dma_act(nc, st, sT, mc * MS, MS)
                for dt in range(CT):
                    ps = kps.tile([128, MS], F32, tag="kps")
                    for ct in range(CT):
                        nc.tensor.matmul(
                            ps[:],
                            lhsT=wk[:, dt * C + ct * 128: dt * C + (ct + 1) * 128],
                            rhs=st[:, ct, :],
                            start=(ct == 0),
                            stop=(ct == CT - 1),
                        )
                    nc.scalar.copy(
                        kT[:, dt * M + mc * MS: dt * M + (mc + 1) * MS], ps[:]
                    )

        wk_ctx.close()

        # ---------------- attention: per n-chunk qT, then s/p/o ------------
        # wq is dt-major like wk
        with (
            tc.tile_pool(name="wqp", bufs=1) as wqp,
            tc.tile_pool(name="xq", bufs=1) as xqp,
            tc.tile_pool(name="qt", bufs=1) as qtp,
            tc.tile_pool(name="qps", bufs=2, space="PSUM") as qps,
            tc.tile_pool(name="sps", bufs=2, space="PSUM") as sps,
            tc.tile_pool(name="ptps", bufs=2, space="PSUM") as ptps,
            tc.tile_pool(name="ops", bufs=1, space="PSUM") as ops,
            tc.tile_pool(name="psb", bufs=2) as psbp,
            tc.tile_pool(name="ptsb", bufs=2) as ptsbp,
            tc.tile_pool(name="osb", bufs=2) as osbp,
            tc.tile_pool(name="stat", bufs=4) as statp,
        ):
            wq = wqp.tile([128, CT * C], F32R, tag="wq")
            _dma_w(nc, wq, wqT, 0, 2)  # dt0 block: first qT group's weights
            xq0 = xqp.tile([128, CT, NCHUNK], F32R, tag="xq")
            _dma_act(nc, xq0, xT, 0, NCHUNK)
            _dma_w(nc, wq, wqT, 2, WCH)
            for nch in range(N // NCHUNK):
                if nch == 0:
                    xq = xq0
                else:
                    xq = xqp.tile([128, CT, NCHUNK], F32R, tag="xq")
                    _dma_act(nc, xq, xT, nch * NCHUNK, NCHUNK)
                qt = qtp.tile([128, CT * NCHUNK], F32R, tag="qt")
                for dt in range(CT):
                    ps = qps.tile([128, NCHUNK], F32, tag="qps")
                    for ct in range(CT):
                        nc.tensor.matmul(
                            ps[:],
                            lhsT=wq[:, dt * C + ct * 128: dt * C + (ct + 1) * 128],
                            rhs=xq[:, ct, :],
                            start=(ct == 0),
                            stop=(ct == CT - 1),
                        )
                    nc.scalar.copy(
                        qt[:, dt * NCHUNK:(dt + 1) * NCHUNK], ps[:]
                    )
                for nt2 in range(NCHUNK // 128):
                    ntile = nch * (NCHUNK // 128) + nt2
                    partials = statp.tile([128, 4], F32, tag="partials")
                    o_ps = ops.tile([128, C], F32, tag="ops")
                    for g in range(M // MS):
                        s_ps = sps.tile([128, MS], F32, tag="sps")
                        for dt in range(CT):
                            nc.tensor.matmul(
                                s_ps[:],
                                lhsT=qt[:, dt * NCHUNK + nt2 * 128: dt * NCHUNK + (nt2 + 1) * 128],
                                rhs=kT[:, dt * M + g * MS: dt * M + (g + 1) * MS],
                                start=(dt == 0),
                                stop=(dt == CT - 1),
                            )
                        p_sb = psbp.tile([128, MS], F32R, tag="psb")
                        nc.scalar.activation(
                            p_sb[:], s_ps[:], AF.Exp,
                            scale=float(SCALE),
                            accum_out=partials[:, g:g + 1],
                        )
                        pt_ps = ptps.tile([128, MS], F32R, tag="ptps")
                        for j in range(MS // 128):
                            nc.tensor.transpose(
                                pt_ps[:, j * 128:(j + 1) * 128],
                                p_sb[:, j * 128:(j + 1) * 128],
                                ident[:],
                            )
                        pt_sb = ptsbp.tile([128, MS], F32R, tag="ptsb")
                        nc.vector.tensor_copy(pt_sb[:], pt_ps[:])
                        for j in range(MS // 128):
                            mt = g * (MS // 128) + j
                            for dc in range(C // 512):
                                nc.tensor.matmul(
                                    o_ps[:, dc * 512:(dc + 1) * 512],
                                    lhsT=pt_sb[:, j * 128:(j + 1) * 128],
                                    rhs=v[:, mt * C + dc * 512: mt * C + (dc + 1) * 512],
                                    start=(mt == 0),
                                    stop=(mt == MT - 1),
                                )
                    denom = statp.tile([128, 1], F32, tag="denom")
                    nc.vector.reduce_sum(
                        denom[:], partials[:], axis=mybir.AxisListType.X
                    )
                    recip = statp.tile([128, 1], F32, tag="recip")
                    nc.vector.reciprocal(recip[:], denom[:])
                    o_sb = osbp.tile([128, C], F32, tag="osb")
                    nc.vector.tensor_scalar_mul(o_sb[:], o_ps[:], recip[:])
                    nc.sync.dma_start(
                        o_dram[ntile * 128:(ntile + 1) * 128, :], o_sb[:]
                    )

        kv_ctx.close()

        # ---------------- projection with the swapaxes/reshape fold --------
        # pw is dc-major like wv
        with (
            tc.tile_pool(name="pwp", bufs=1) as pwp,
            tc.tile_pool(name="bp", bufs=1) as bp,
            tc.tile_pool(name="obp", bufs=2) as obp,
            tc.tile_pool(name="fps", bufs=2, space="PSUM") as fps,
            tc.tile_pool(name="fsb", bufs=2) as fsbp,
        ):
            pw = pwp.tile([128, CT * C], F32R, tag="pw")
            bias = bp.tile([128, C], F32, tag="bias")
            ob0 = obp.tile([128, CT * C], F32R, tag="ob")
            cw = (CT * C) // WCH
            for i in range(CT):
                nc.sync.dma_start(
                    pw[:, i * cw:(i + 1) * cw],
                    pwT[:, i * cw:(i + 1) * cw].bitcast(F32R),
                )
                # plain 2D slices: a rearranged AP on a DRAM pool tile defeats
                # Tile's RAW dep tracking (read would race the o_dram writes)
                nc.sync.dma_start(
                    ob0[:, i * C:(i + 1) * C],
                    o_dram[i * 128:(i + 1) * 128, :].bitcast(F32R),
                )
            _dma_w(nc, pw, pwT, CT, WCH)
            nc.sync.dma_start(bias[:], biasb[:])
            out_v = out[:].rearrange("(t two) d -> two t d", two=2)
            for i in range(2):
                if i == 0:
                    ob = ob0
                else:
                    ob = obp.tile([128, CT * C], F32R, tag="ob")
                    for ct in range(CT):
                        nc.sync.dma_start(
                            ob[:, ct * C:(ct + 1) * C],
                            o_dram[i * C + ct * 128: i * C + (ct + 1) * 128, :].bitcast(F32R),
                        )
                for dc in range(C // 512):
                    for tt in range(CT):
                        ps = fps.tile([128, 512], F32, tag="fps")
                        for ct in range(CT):
                            nc.tensor.matmul(
                                ps[:],
                                lhsT=ob[:, ct * C + tt * 128: ct * C + (tt + 1) * 128],
                                rhs=pw[:, dc * 4096 + ct * 512: dc * 4096 + (ct + 1) * 512],
                                start=(ct == 0),
                                stop=(ct == CT - 1),
                            )
                        f_sb = fsbp.tile([128, 512], F32, tag="fsb")
                        nc.vector.tensor_add(
                            f_sb[:], ps[:], bias[:, dc * 512:(dc + 1) * 512]
                        )
                        nc.sync.dma_start(
                            out_v[i, tt * 128:(tt + 1) * 128, dc * 512:(dc + 1) * 512],
                            f_sb[:],
                        )


def _prep_w_lhs(w):
    # lhsT weights (wk, wq): dt-major [p, dt*1024 + ct*128 + dd]
    wt = w.T.reshape(CT, 128, CT, 128)          # [ct, p, dt, dd]
    return np.ascontiguousarray(
        wt.transpose(1, 2, 0, 3).reshape(128, CT * C)
    )


def _prep_w_rhs(w):
    # rhs weights (wv, pw): dc-major [p, dc*4096 + ct*512 + dd]
    wt = w.T.reshape(CT, 128, C // 512, 512)    # [ct, p, dc, dd]
    return np.ascontiguousarray(
        wt.transpose(1, 2, 0, 3).reshape(128, CT * C)
    )


def _prep_act(a):
    # a [rows, C] -> a.T [C, rows] grouped as [p, ct, rows]
    n = a.shape[0]
    return np.ascontiguousarray(a.T.reshape(CT, 128, n).transpose(1, 0, 2))


def prep_in_maps(x, support, attn_mask, qkv_w, proj_w, proj_b):
    x = np.asarray(x, dtype=np.float32)
    support = np.asarray(support, dtype=np.float32)
    attn_mask = np.asarray(attn_mask)
    qkv_w = np.asarray(qkv_w, dtype=np.float32)
    proj_w = np.asarray(proj_w, dtype=np.float32)
    proj_b = np.asarray(proj_b, dtype=np.float32)

    wq = _prep_w_lhs(qkv_w[:C])
    wk = _prep_w_lhs(qkv_w[C:2 * C])
    wv = _prep_w_rhs(qkv_w[2 * C:])
    pw = _prep_w_rhs(proj_w)
    maskf = np.ascontiguousarray(
        attn_mask.astype(np.float32).reshape(MT, 128).T
    )
    biasb = np.ascontiguousarray(np.broadcast_to(proj_b, (128, C)))

    in_maps = []
    for b in range(B):
        in_maps.append({
            "xT": _prep_act(x[b]),
            "sT": _prep_act(support[b]),
            "wqT": wq,
            "wkT": wk,
            "wvT": wv,
            "pwT": pw,
            "maskf": maskf,
            "biasb": biasb,
        })
    return in_maps


def kernel(x, support, attn_mask, qkv_w, proj_w, proj_b):
    if "nc" not in _CACHE:
        _CACHE["nc"] = _build_program()
    nc = _CACHE["nc"]

    in_maps = prep_in_maps(x, support, attn_mask, qkv_w, proj_w, proj_b)
    res = run_bass_kernel_spmd(nc, in_maps, core_ids=list(range(B)))
    return np.stack([res.results[b]["out"] for b in range(B)], axis=0)



# revision 6
# speedup vs baseline: 1.3007x; 1.3007x over previous
"""Trainium2 Bass kernel for CrossAttention (B=8, N=M=2048, C=1024), fp32.

Sharding: data-parallel — one batch element per NeuronCore (8 cores).

Per-core computation (batch b), with a host-side permutation of the support
rows that places the attn_mask==1 ("masked-in") rows first:
  qT[d, n] = (x @ wq^T)^T                   (chunks 0..1 built up front — the
                                             phase needs only ~1.5MB of DMA
                                             before the PE can start)
  kT[d, m] = (support_perm @ wk^T)^T        (all M rows — softmax denominator
                                             spans every support position)
  v[m, d]  = (support_perm @ wv^T) * mask   (ONLY the first n_vt tiles of 128
                                             rows; masked-out rows contribute
                                             nothing to p@v so they are never
                                             built and never contracted)
  p = exp(SCALE * q k^T)   (no max subtraction: logits ~ N(0, 8), safe fp32;
                            per-row sums accumulate the softmax denominator)
  o[n, d] = (p[:, :n_vt*128] @ v) / rowsum_all(p)
  The reference's  out.swapaxes(1,2).reshape(N, C)  permutation satisfies
  o_perm[2t+i, c] = o[1024*i + c, t], so the final projection becomes
  final[2t+i, d'] = sum_c o[1024*i + c, t] * proj_w[d', c]  — a plain matmul
  with o-block-i rows as the contraction dim, written out with row stride 2.

fp32r operands (full-rate PE streaming) for everything feeding the softmax
logits; bf16 for the p/v/o/proj operands where quantization error (~0.4%)
is far inside the 2e-2 gate — bf16 drops the PE transpose cost to 1.0
cyc/row and lets v + o + pw all stay SBUF-resident (no DRAM bounce before
projection).  The attention inner loop is software-pipelined (s of group
g+1 issues before the transpose/pv of group g) so the PE never idles
waiting on exp (Act) or the psum->sbuf copies (DVE).
"""

import sys

sys.path.insert(0, "/opt/trn_rl_repo")

import numpy as np
import ml_dtypes

import concourse.bass as bass
import concourse.tile as tile
from concourse import bacc, mybir
from concourse.bass_utils import run_bass_kernel_spmd
from concourse.masks import make_identity

F32 = mybir.dt.float32
F32R = mybir.dt.float32r
BF16 = mybir.dt.bfloat16
AF = mybir.ActivationFunctionType

B, N, M, C = 8, 2048, 2048, 1024
CT = C // 128          # 8 c-tiles (contraction / channel partition tiles)
MT = M // 128          # 16 m-tiles
SCALE = (C // 8) ** -0.5
NCH = 512              # attention n-chunk (qt tile width)
QS = 256               # x load / q psum sub-chunk
KS = 256               # VK phase m-chunk
MS = 512               # attention m-group (s matmul free dim)

_CACHE = {}


def _build_program(n_vt):
    nc = bacc.Bacc(
        "TRN2",
        target_bir_lowering=False,
        debug=False,
        enable_asserts=False,
        num_devices=8,
    )

    xT = nc.dram_tensor("xT", [128, CT, N], F32, kind="ExternalInput")
    sT = nc.dram_tensor("sT", [128, CT, M], F32, kind="ExternalInput")
    wqT = nc.dram_tensor("wqT", [128, CT * C], F32, kind="ExternalInput")
    wkT = nc.dram_tensor("wkT", [128, CT * C], F32, kind="ExternalInput")
    wvT = nc.dram_tensor("wvT", [128, CT * C], F32, kind="ExternalInput")
    pwB = nc.dram_tensor("pwB", [128, CT * C], BF16, kind="ExternalInput")
    maskf = nc.dram_tensor("maskf", [128, n_vt], F32, kind="ExternalInput")
    biasb = nc.dram_tensor("biasb", [128, C], F32, kind="ExternalInput")
    out = nc.dram_tensor("out", [N, C], F32, kind="ExternalOutput")

    with tile.TileContext(nc, pool_alloc_mode="queue") as tc:
        _trace_kernel(tc, n_vt, xT, sT, wqT, wkT, wvT, pwB, maskf, biasb, out)
    nc.compile()
    return nc


def _dma_act(nc, atile, adram, col0, cols):
    # per-ct chunked activation load (arrival order == psum-group order)
    for ct in range(CT):
        nc.sync.dma_start(
            atile[:, ct, :],
            adram[:, ct, col0:col0 + cols].bitcast(F32R),
        )


def _q_build(nc, qps, wq, xq, qt, sub):
    # qt[:, dt*NCH + off : off+QS] = (x-chunk @ wq^T)^T for one QS sub-chunk
    for dt in range(CT):
        ps = qps.tile([128, QS], F32, tag="qps")
        for ct in range(CT):
            nc.tensor.matmul(
                ps[:],
                lhsT=wq[:, dt * C + ct * 128: dt * C + (ct + 1) * 128],
                rhs=xq[:, ct, :],
                start=(ct == 0),
                stop=(ct == CT - 1),
            )
        nc.scalar.copy(qt[:, dt * NCH + sub * QS: dt * NCH + (sub + 1) * QS], ps[:])


def _load_wq(nc, wq, wqT, first_chunk_only=False):
    cw = (CT * C) // 8
    rng = range(1) if first_chunk_only else range(1, CT)
    for i in rng:
        nc.sync.dma_start(
            wq[:, i * cw:(i + 1) * cw],
            wqT[:, i * cw:(i + 1) * cw].bitcast(F32R),
        )


def _trace_kernel(tc, n_vt, xT, sT, wqT, wkT, wvT, pwB, maskf, biasb, out):
    nc = tc.nc

    from contextlib import ExitStack

    n_pre = 2 if n_vt <= 12 else 1   # qT chunks built before VK (SBUF budget)

    with ExitStack() as ctx:
        persist = ctx.enter_context(tc.tile_pool(name="persist", bufs=1))

        ident0 = persist.tile([128, 128], F32, tag="ident0")
        make_identity(nc, ident0[:])
        identb = persist.tile([128, 128], BF16, tag="identb")
        nc.scalar.copy(identb[:], ident0[:])
        maskt = persist.tile([128, n_vt], F32, tag="maskt")
        nc.sync.dma_start(maskt[:], maskf[:])

        # ---- long-lived operands -----------------------------------------
        kv_ctx = ctx.enter_context(ExitStack())
        kTp = kv_ctx.enter_context(tc.tile_pool(name="kTp", bufs=1))
        kT = kTp.tile([128, CT * M], F32R, tag="kT")      # 8 MB
        vp = kv_ctx.enter_context(tc.tile_pool(name="vp", bufs=1))
        v = vp.tile([128, n_vt * C], BF16, tag="v")       # 2 MB @ n_vt=8
        qtp = ctx.enter_context(tc.tile_pool(name="qtp", bufs=2))

        # ---------------- phase Q: qT for the first n_pre n-chunks --------
        # wq is dt-major: the first q group needs only wq cols [0:1024]
        qts = []
        with (
            tc.tile_pool(name="wqp", bufs=1) as wqp,
            tc.tile_pool(name="xqp", bufs=2) as xqp,
            tc.tile_pool(name="qps", bufs=2, space="PSUM") as qps,
        ):
            wq = wqp.tile([128, CT * C], F32R, tag="wq")
            for nch in range(n_pre):
                qt = qtp.tile([128, CT * NCH], F32R, tag="qt")
                qts.append(qt)
                for sub in range(2):
                    xq = xqp.tile([128, CT, QS], F32R, tag="xq")
                    if nch == 0 and sub == 0:
                        _load_wq(nc, wq, wqT, first_chunk_only=True)
                        _dma_act(nc, xq, xT, 0, QS)
                        _load_wq(nc, wq, wqT)
                    else:
                        _dma_act(nc, xq, xT, nch * NCH + sub * QS, QS)
                    _q_build(nc, qps, wq, xq, qt, sub)

        # ---------------- phase VK: kT (all m) + v (masked-in tiles) -------
        # wk is dt-major; wv is dc-major
        with (
            tc.tile_pool(name="wkp", bufs=1) as wkp,
            tc.tile_pool(name="wvp", bufs=1) as wvp,
            tc.tile_pool(name="stp", bufs=2) as stp,
            tc.tile_pool(name="kps", bufs=2, space="PSUM") as kps,
            tc.tile_pool(name="vps", bufs=2, space="PSUM") as vps,
        ):
            cw = (CT * C) // 8
            wk = wkp.tile([128, CT * C], F32R, tag="wk")
            wv = wvp.tile([128, CT * C], F32R, tag="wv")
            for i in range(CT):
                nc.sync.dma_start(
                    wk[:, i * cw:(i + 1) * cw],
                    wkT[:, i * cw:(i + 1) * cw].bitcast(F32R),
                )
            for i in range(4):
                nc.sync.dma_start(
                    wv[:, i * 2048:(i + 1) * 2048],
                    wvT[:, i * 2048:(i + 1) * 2048].bitcast(F32R),
                )
            for mc in range(M // KS):
                st = stp.tile([128, CT, KS], F32R, tag="st")
                _dma_act(nc, st, sT, mc * KS, KS)
                for dt in range(CT):
                    ps = kps.tile([128, KS], F32, tag="kps")
                    for ct in range(CT):
                        nc.tensor.matmul(
                            ps[:],
                            lhsT=wk[:, dt * C + ct * 128: dt * C + (ct + 1) * 128],
                            rhs=st[:, ct, :],
                            start=(ct == 0),
                            stop=(ct == CT - 1),
                        )
                    nc.scalar.copy(
                        kT[:, dt * M + mc * KS: dt * M + (mc + 1) * KS], ps[:]
                    )
                for j in range(KS // 128):
                    mt = mc * (KS // 128) + j
                    if mt >= n_vt:
                        continue
                    for dc in range(C // 512):
                        ps = vps.tile([128, 512], F32, tag="vps")
                        for ct in range(CT):
                            nc.tensor.matmul(
                                ps[:],
                                lhsT=st[:, ct, j * 128:(j + 1) * 128],
                                rhs=wv[:, dc * 4096 + ct * 512: dc * 4096 + (ct + 1) * 512],
                                start=(ct == 0),
                                stop=(ct == CT - 1),
                            )
                        nc.vector.tensor_scalar_mul(
                            v[:, mt * C + dc * 512: mt * C + (dc + 1) * 512],
                            ps[:],
                            maskt[:, mt:mt + 1],
                        )

        # ---------------- attention: pipelined s / exp / T / pv ------------
        op = ctx.enter_context(tc.tile_pool(name="op", bufs=1))
        o_all = op.tile([128, N // 128 * C], BF16, tag="o_all")  # 4 MB

        q2_ctx = ExitStack()   # reloaded wq for the late q builds
        late_ctx = ExitStack()  # pw + bias, opened once wq's space frees
        try:
            wqp2 = q2_ctx.enter_context(
                tc.tile_pool(name="wqp2", bufs=1, side="right"))
            xqp2 = q2_ctx.enter_context(
                tc.tile_pool(name="xqp2", bufs=2, side="right"))
            wq2 = wqp2.tile([128, CT * C], F32R, tag="wq2")
            _load_wq(nc, wq2, wqT, first_chunk_only=True)
            _load_wq(nc, wq2, wqT)

            with (
                tc.tile_pool(name="sps", bufs=2, space="PSUM") as sps,
                tc.tile_pool(name="ptps", bufs=2, space="PSUM") as ptps,
                tc.tile_pool(name="ops", bufs=1, space="PSUM") as ops,
                tc.tile_pool(name="psb", bufs=2) as psbp,
                tc.tile_pool(name="ptsb", bufs=2) as ptsbp,
                tc.tile_pool(name="stat", bufs=4) as statp,
            ):
                # qps2 on top of the PSUM stack: released at the last q build
                qps2 = q2_ctx.enter_context(
                    tc.tile_pool(name="qps2", bufs=2, space="PSUM"))
                NG = M // MS          # 4 m-groups per n-tile
                for nch in range(N // NCH):
                    if nch >= n_pre:
                        # build this chunk's qT (wq2/x prefetched earlier)
                        qt = qtp.tile([128, CT * NCH], F32R, tag="qt")
                        qts.append(qt)
                        for sub in range(2):
                            xq = xqp2.tile([128, CT, QS], F32R, tag="xq2")
                            _dma_act(nc, xq, xT, nch * NCH + sub * QS, QS)
                            _q_build(nc, qps2, wq2, xq, qt, sub)
                    qt = qts[nch]
                    if nch == N // NCH - 1:
                        # last q build done: wq/xq space frees; land pw+bias
                        q2_ctx.close()
                        pwp = late_ctx.enter_context(
                            tc.tile_pool(name="pwp", bufs=1, side="right"))
                        pw = pwp.tile([128, CT * C], BF16, tag="pw")
                        bias = pwp.tile([128, C], F32, tag="bias")
                        for i in range(4):
                            nc.sync.dma_start(
                                pw[:, i * 2048:(i + 1) * 2048],
                                pwB[:, i * 2048:(i + 1) * 2048],
                            )
                        nc.sync.dma_start(bias[:], biasb[:])

                    for nt2 in range(NCH // 128):
                        ntile = nch * (NCH // 128) + nt2
                        partials = statp.tile([128, NG], F32, tag="partials")
                        o_ps = ops.tile([128, C], F32, tag="ops")

                        # one-group software pipeline: s-matmuls of group
                        # g+1 issue before transpose/pv of group g, hiding
                        # exp (Act) and the psum->sbuf copy (DVE) behind PE
                        # work.
                        pend = None  # (p_sb, g)
                        for g in range(NG):
                            s_ps = sps.tile([128, MS], F32, tag="sps")
                            for dt in range(CT):
                                nc.tensor.matmul(
                                    s_ps[:],
                                    lhsT=qt[:, dt * NCH + nt2 * 128: dt * NCH + (nt2 + 1) * 128],
                                    rhs=kT[:, dt * M + g * MS: dt * M + (g + 1) * MS],
                                    start=(dt == 0),
                                    stop=(dt == CT - 1),
                                )
                            p_sb = psbp.tile([128, MS], BF16, tag="psb")
                            nc.scalar.activation(
                                p_sb[:], s_ps[:], AF.Exp,
                                scale=float(SCALE),
                                accum_out=partials[:, g:g + 1],
                            )
                            if pend is not None:
                                _transpose_pv(nc, ptps, ptsbp, identb, v,
                                              o_ps, *pend, n_vt)
                            pend = (p_sb, g)
                        _transpose_pv(nc, ptps, ptsbp, identb, v, o_ps,
                                      *pend, n_vt)

                        denom = statp.tile([128, 1], F32, tag="denom")
                        nc.vector.reduce_sum(
                            denom[:], partials[:], axis=mybir.AxisListType.X
                        )
                        recip = statp.tile([128, 1], F32, tag="recip")
                        nc.vector.reciprocal(recip[:], denom[:])
                        nc.vector.tensor_scalar_mul(
                            o_all[:, ntile * C:(ntile + 1) * C], o_ps[:], recip[:]
                        )

            # ---------------- projection with the swapaxes/reshape fold ----
            with (
                tc.tile_pool(name="fps", bufs=2, space="PSUM") as fps,
                tc.tile_pool(name="fsb", bufs=3) as fsbp,
            ):
                out_v = out[:].rearrange("(t two) d -> two t d", two=2)
                for i in range(2):
                    for dc in range(C // 512):
                        for tt in range(CT):
                            ps = fps.tile([128, 512], F32, tag="fps")
                            for ct in range(CT):
                                nc.tensor.matmul(
                                    ps[:],
                                    lhsT=o_all[:, (i * CT + ct) * C + tt * 128: (i * CT + ct) * C + (tt + 1) * 128],
                                    rhs=pw[:, dc * 4096 + ct * 512: dc * 4096 + (ct + 1) * 512],
                                    start=(ct == 0),
                                    stop=(ct == CT - 1),
                                )
                            f_sb = fsbp.tile([128, 512], F32, tag="fsb")
                            nc.vector.tensor_add(
                                f_sb[:], ps[:], bias[:, dc * 512:(dc + 1) * 512]
                            )
                            nc.sync.dma_start(
                                out_v[i, tt * 128:(tt + 1) * 128, dc * 512:(dc + 1) * 512],
                                f_sb[:],
                            )
        finally:
            late_ctx.close()
            q2_ctx.close()


def _transpose_pv(nc, ptps, ptsbp, identb, v, o_ps, p_sb, g, n_vt):
    # transpose the masked-in 128-tiles of p group g, then accumulate p@v
    mts = [mt for mt in range(g * (MS // 128), (g + 1) * (MS // 128))
           if mt < n_vt]
    if not mts:
        return
    pt_ps = ptps.tile([128, 128 * len(mts)], BF16, tag="ptps")
    for idx, mt in enumerate(mts):
        j = mt - g * (MS // 128)
        nc.tensor.transpose(
            pt_ps[:, idx * 128:(idx + 1) * 128],
            p_sb[:, j * 128:(j + 1) * 128],
            identb[:],
        )
    pt_sb = ptsbp.tile([128, 128 * len(mts)], BF16, tag="ptsb")
    nc.vector.tensor_copy(pt_sb[:], pt_ps[:])
    for idx, mt in enumerate(mts):
        for dc in range(C // 512):
            nc.tensor.matmul(
                o_ps[:, dc * 512:(dc + 1) * 512],
                lhsT=pt_sb[:, idx * 128:(idx + 1) * 128],
                rhs=v[:, mt * C + dc * 512: mt * C + (dc + 1) * 512],
                start=(mt == 0),
                stop=(mt == n_vt - 1),
            )


def _prep_w_lhs(w):
    # lhsT weights (wk, wq): dt-major [p, dt*1024 + ct*128 + dd]
    wt = w.T.reshape(CT, 128, CT, 128)          # [ct, p, dt, dd]
    return np.ascontiguousarray(
        wt.transpose(1, 2, 0, 3).reshape(128, CT * C)
    )


def _prep_w_rhs(w):
    # rhs weights (wv, pw): dc-major [p, dc*4096 + ct*512 + dd]
    wt = w.T.reshape(CT, 128, C // 512, 512)    # [ct, p, dc, dd]
    return np.ascontiguousarray(
        wt.transpose(1, 2, 0, 3).reshape(128, CT * C)
    )


def _prep_act(a):
    # a [rows, C] -> a.T [C, rows] grouped as [p, ct, rows]
    n = a.shape[0]
    return np.ascontiguousarray(a.T.reshape(CT, 128, n).transpose(1, 0, 2))


def prep_in_maps(x, support, attn_mask, qkv_w, proj_w, proj_b):
    x = np.asarray(x, dtype=np.float32)
    support = np.asarray(support, dtype=np.float32)
    attn_mask = np.asarray(attn_mask)
    qkv_w = np.asarray(qkv_w, dtype=np.float32)
    proj_w = np.asarray(proj_w, dtype=np.float32)
    proj_b = np.asarray(proj_b, dtype=np.float32)

    # host permutation: masked-in support rows first (stable order)
    mk = attn_mask.astype(bool)
    order = np.argsort(~mk, kind="stable")
    n_in = int(mk.sum())
    n_vt = max(1, -(-n_in // 128))
    maskp = mk[order].astype(np.float32)
    maskf = np.ascontiguousarray(maskp[:n_vt * 128].reshape(n_vt, 128).T)

    wq = _prep_w_lhs(qkv_w[:C])
    wk = _prep_w_lhs(qkv_w[C:2 * C])
    wv = _prep_w_rhs(qkv_w[2 * C:])
    pw = _prep_w_rhs(proj_w).astype(ml_dtypes.bfloat16)
    biasb = np.ascontiguousarray(np.broadcast_to(proj_b, (128, C)))

    sperm = support[:, order, :]

    in_maps = []
    for b in range(B):
        in_maps.append({
            "xT": _prep_act(x[b]),
            "sT": _prep_act(sperm[b]),
            "wqT": wq,
            "wkT": wk,
            "wvT": wv,
            "pwB": pw,
            "maskf": maskf,
            "biasb": biasb,
        })
    return in_maps, n_vt


def kernel(x, support, attn_mask, qkv_w, proj_w, proj_b):
    in_maps, n_vt = prep_in_maps(x, support, attn_mask, qkv_w, proj_w, proj_b)
    key = ("nc", n_vt)
    if key not in _CACHE:
        _CACHE[key] = _build_program(n_vt)
    nc = _CACHE[key]
    _CACHE["nc"] = nc

    res = run_bass_kernel_spmd(nc, in_maps, core_ids=list(range(B)))
    return np.stack([res.results[b]["out"] for b in range(B)], axis=0)


# revision 30
# speedup vs baseline: 1.5588x; 1.1985x over previous
"""Trainium2 Bass kernel for CrossAttention (B=8, N=M=2048, C=1024), fp32.

Sharding: data-parallel — one batch element per NeuronCore (8 cores).

Key algebraic fold: the module never reshapes into heads, so
  s = (x wq^T)(support wk^T)^T = x (wq^T wk) support^T.
W = wq^T wk is precomputed exactly on the host, which deletes the entire
k-projection GEMM (and its 8MB kT buffer) from the device: the kernel
computes xw = x @ W once, and the s-matmul contracts xw directly against
support^T — which is already the DMA input layout.

Per-core computation (batch b), with a host-side permutation of the support
rows that places the attn_mask==1 ("masked-in") rows first:
  xwT[c', n] = (x @ W)^T                    (all n up front; PE-bound phase)
  v[m, d]  = (support_perm @ wv^T) * mask   (ONLY the first n_vt tiles of 128
                                             rows; masked-out rows contribute
                                             nothing to p@v so they are never
                                             built and never contracted)
  p = exp(SCALE * xw support^T)  (no max subtraction: logits ~ N(0, 8);
                                  per-row sums give the softmax denominator,
                                  which spans ALL support positions)
  o[n, d] = (p[:, :n_vt*128] @ v) / rowsum_all(p)
  The reference's  out.swapaxes(1,2).reshape(N, C)  permutation satisfies
  o_perm[2t+i, c] = o[1024*i + c, t], so the final projection becomes
  final[2t+i, d'] = sum_c o[1024*i + c, t] * proj_w[d', c]  — a plain matmul
  with o-block-i rows as the contraction dim, written out with row stride 2.

fp32r operands (full-rate PE streaming) for everything feeding the softmax
logits; bf16 for the p/v/o/proj operands where quantization error (~0.4%)
is far inside the 2e-2 gate — bf16 drops the PE transpose cost to 1.0
cyc/row and lets v + o + pw all stay SBUF-resident (no DRAM bounce).

Scheduling notes:
 - Tile-pool alloc boundaries serialize against the prior phase's readers
   of the reused space, so a fresh pool's DMAs can't prefetch.  PSUM pools
   and the big resident tiles (sT on the right allocator side, xw, v, o)
   are opened once; only W/xq (phase XW) and wv (phase V) are transient.
 - The attention inner loop is software-pipelined (s of group g+1 issues
   before transpose/pv of group g) so the PE never waits on exp (Act) or
   the psum->sbuf copies (DVE).
 - An idle PE resets the 0.65/1.2/2.4 GHz p-state ramp (~+1us per stall).
   A warm-up matmul chain covers the initial DMA wait, and filler matmuls
   pad the known DMA-gated windows (W stream at start, wv at phase V).
 - DMA transfers are batched (>=0.5MB strided slabs): per-transfer
   overhead is ~0.4us.
"""

import sys

sys.path.insert(0, "/opt/trn_rl_repo")

import numpy as np
import ml_dtypes

import concourse.bass as bass
import concourse.tile as tile
from concourse import bacc, mybir
from concourse.bass_utils import run_bass_kernel_spmd
from concourse.masks import make_identity

F32 = mybir.dt.float32
F32R = mybir.dt.float32r
BF16 = mybir.dt.bfloat16
AF = mybir.ActivationFunctionType

B, N, M, C = 8, 2048, 2048, 1024
CT = C // 128          # 8 c-tiles (contraction / channel partition tiles)
MT = M // 128          # 16 m-tiles
SCALE = (C // 8) ** -0.5
QS = 256               # x load / xw psum sub-chunk
MS = 512               # attention m-group (s matmul free dim)
WARMUP_MM = 36         # PE p-state warm-up matmuls during startup DMA

_CACHE = {}


def _build_program(n_vt):
    nc = bacc.Bacc(
        "TRN2",
        target_bir_lowering=False,
        debug=False,
        enable_asserts=False,
        num_devices=8,
    )

    xT = nc.dram_tensor("xT", [128, CT, N], F32, kind="ExternalInput")
    sT = nc.dram_tensor("sT", [128, CT, M], F32, kind="ExternalInput")
    wT = nc.dram_tensor("wT", [128, CT * C], F32, kind="ExternalInput")
    wvT = nc.dram_tensor("wvT", [128, CT * C], F32, kind="ExternalInput")
    pwB = nc.dram_tensor("pwB", [128, CT * C], BF16, kind="ExternalInput")
    maskf = nc.dram_tensor("maskf", [128, n_vt], F32, kind="ExternalInput")
    biasb = nc.dram_tensor("biasb", [128, C], F32, kind="ExternalInput")
    out = nc.dram_tensor("out", [N, C], F32, kind="ExternalOutput")

    with tile.TileContext(nc, pool_alloc_mode="queue") as tc:
        _trace_kernel(tc, n_vt, xT, sT, wT, wvT, pwB, maskf, biasb, out)
    nc.compile()
    return nc


def _dma_act(nc, atile, adram, col0, cols, split=1):
    # batched strided load (per-transfer overhead ~0.4us); split>1 only for
    # the very first load where startup latency matters
    step = CT // split
    for i in range(split):
        nc.sync.dma_start(
            atile[:, i * step:(i + 1) * step, :],
            adram[:, i * step:(i + 1) * step, col0:col0 + cols].bitcast(F32R),
        )


def _trace_kernel(tc, n_vt, xT, sT, wT, wvT, pwB, maskf, biasb, out):
    nc = tc.nc

    from contextlib import ExitStack

    with ExitStack() as ctx:
        persist = ctx.enter_context(tc.tile_pool(name="persist", bufs=1))

        # program-lifetime PSUM pools, shared across phases by tag (slots
        # are bank-granular; fresh per-phase pools add boundary stalls)
        p512 = ctx.enter_context(tc.tile_pool(name="p512", bufs=2,
                                              space="PSUM"))
        p256 = ctx.enter_context(tc.tile_pool(name="p256", bufs=2,
                                              space="PSUM"))
        ptps = ctx.enter_context(tc.tile_pool(name="ptp", bufs=2,
                                              space="PSUM"))
        ops = ctx.enter_context(tc.tile_pool(name="pop", bufs=1,
                                             space="PSUM"))

        # p-state warm-up: memset a scratch tile (no input deps), then a
        # chained matmul run keeps the PE busy through the initial DMA wait
        wub = persist.tile([128, 256], F32, tag="wub")
        nc.vector.memset(wub[:], 1.0)

        def filler(n):
            # f32r is a PE-side view: memset must target a plain f32 tile
            # (f32r memset fails the neuronxcc ISA check), bitcast at use
            for _ in range(n):
                ps = p256.tile([128, 256], F32, tag="g256")
                nc.tensor.matmul(ps[:], lhsT=wub[:, 0:128].bitcast(F32R),
                                 rhs=wub[:].bitcast(F32R),
                                 start=True, stop=True)

        filler(WARMUP_MM)

        ident0 = persist.tile([128, 128], F32, tag="ident0")
        make_identity(nc, ident0[:])
        identb = persist.tile([128, 128], BF16, tag="identb")
        nc.scalar.copy(identb[:], ident0[:])
        maskt = persist.tile([128, n_vt], F32, tag="maskt")
        nc.sync.dma_start(maskt[:], maskf[:])

        # ---- long-lived operands -----------------------------------------
        # support^T resident on the right allocator side: it is both the
        # s-matmul rhs and the v-build lhsT, and releases before proj
        sT_ctx = ExitStack()
        sTp = sT_ctx.enter_context(tc.tile_pool(name="sTp", bufs=1,
                                                side="right"))
        sTr = sTp.tile([128, CT, M], F32R, tag="sTr")     # 8 MB
        vp = ctx.enter_context(tc.tile_pool(name="vp", bufs=1))
        v = vp.tile([128, n_vt * C], BF16, tag="v")       # 2 MB @ n_vt=8
        xwp = ctx.enter_context(tc.tile_pool(name="xwp", bufs=1))
        xw = xwp.tile([128, CT * N], F32R, tag="xw")      # 8 MB

        # ---------------- phase XW: (x @ W)^T for all n --------------------
        # W is dt-major: the first group needs only cols [0:1024]
        cw = (CT * C) // 8
        with (
            tc.tile_pool(name="wp", bufs=1) as wp,
            tc.tile_pool(name="xqp", bufs=2) as xqp,
        ):
            w = wp.tile([128, CT * C], F32R, tag="w")
            for sub in range(N // QS):
                xq = xqp.tile([128, CT, QS], F32R, tag="xq")
                first = sub == 0
                if first:
                    nc.sync.dma_start(
                        w[:, 0:cw], wT[:, 0:cw].bitcast(F32R))
                    _dma_act(nc, xq, xT, 0, QS, split=2)
                    for i in range(1, CT):
                        nc.sync.dma_start(
                            w[:, i * cw:(i + 1) * cw],
                            wT[:, i * cw:(i + 1) * cw].bitcast(F32R),
                        )
                else:
                    _dma_act(nc, xq, xT, sub * QS, QS)
                # W streams at ~1.8us/chunk vs 0.85us/group burn in the
                # first sub: pad the supply gaps to avoid p-state resets
                for dt in range(CT):
                    ps = p256.tile([128, QS], F32, tag="g256")
                    for ct in range(CT):
                        nc.tensor.matmul(
                            ps[:],
                            lhsT=w[:, dt * C + ct * 128: dt * C + (ct + 1) * 128],
                            rhs=xq[:, ct, :],
                            start=(ct == 0),
                            stop=(ct == CT - 1),
                        )
                    nc.scalar.copy(
                        xw[:, dt * N + sub * QS: dt * N + (sub + 1) * QS],
                        ps[:],
                    )
                    if first:
                        filler(3)
            # support stream into the already-open right-side pool: the
            # first half (v-build + attention groups 0/1) leads the queue
            for i in range(2):
                nc.sync.dma_start(
                    sTr[:, :, i * 512:(i + 1) * 512],
                    sT[:, :, i * 512:(i + 1) * 512].bitcast(F32R),
                )

        # ---------------- phase V: v[m, d] for the masked-in tiles ---------
        # pw/bias land behind wv and the sT tail in the DMA queue; wv's
        # space frees into W/xq's (boundary on the last xw-build read)
        pwp = ctx.enter_context(tc.tile_pool(name="pwp", bufs=1))
        pw = pwp.tile([128, CT * C], BF16, tag="pw")
        bias = pwp.tile([128, C], F32, tag="bias")
        with tc.tile_pool(name="wvp", bufs=1) as wvp:
            wv = wvp.tile([128, CT * C], F32R, tag="wv")
            for i in range(4):
                nc.sync.dma_start(
                    wv[:, i * 2048:(i + 1) * 2048],
                    wvT[:, i * 2048:(i + 1) * 2048].bitcast(F32R),
                )
            for i in range(2, 4):
                nc.sync.dma_start(
                    sTr[:, :, i * 512:(i + 1) * 512],
                    sT[:, :, i * 512:(i + 1) * 512].bitcast(F32R),
                )
            for i in range(4):
                nc.sync.dma_start(
                    pw[:, i * 2048:(i + 1) * 2048],
                    pwB[:, i * 2048:(i + 1) * 2048],
                )
            nc.sync.dma_start(bias[:], biasb[:])

            filler(16)   # wv streams ~12us behind the phase boundary
            for dc in range(C // 512):
                for mt in range(n_vt):
                    ps = p512.tile([128, 512], F32, tag="g512")
                    for ct in range(CT):
                        nc.tensor.matmul(
                            ps[:],
                            lhsT=sTr[:, ct, mt * 128:(mt + 1) * 128],
                            rhs=wv[:, dc * 4096 + ct * 512: dc * 4096 + (ct + 1) * 512],
                            start=(ct == 0),
                            stop=(ct == CT - 1),
                        )
                    nc.vector.tensor_scalar_mul(
                        v[:, mt * C + dc * 512: mt * C + (dc + 1) * 512],
                        ps[:],
                        maskt[:, mt:mt + 1],
                    )

        # ---------------- attention: pipelined s / exp / T / pv ------------
        psbp = ctx.enter_context(tc.tile_pool(name="psb", bufs=2))
        ptsbp = ctx.enter_context(tc.tile_pool(name="ptsb", bufs=2))
        statp = ctx.enter_context(tc.tile_pool(name="stat", bufs=4))
        op = ctx.enter_context(tc.tile_pool(name="op", bufs=1))
        o_all = op.tile([128, N // 128 * C], BF16, tag="o_all")  # 4 MB

        NG = M // MS          # 4 m-groups per n-tile
        for ntile in range(N // 128):
            partials = statp.tile([128, NG], F32, tag="partials")
            o_ps = ops.tile([128, C], F32, tag="ops")

            # one-group software pipeline: s of group g+1 issues before
            # transpose/pv of group g
            pend = None  # (p_sb, g)
            for g in range(NG):
                s_ps = p512.tile([128, MS], F32, tag="g512")
                for ct in range(CT):
                    nc.tensor.matmul(
                        s_ps[:],
                        lhsT=xw[:, ct * N + ntile * 128: ct * N + (ntile + 1) * 128],
                        rhs=sTr[:, ct, g * MS:(g + 1) * MS],
                        start=(ct == 0),
                        stop=(ct == CT - 1),
                    )
                p_sb = psbp.tile([128, MS], BF16, tag="psb")
                nc.scalar.activation(
                    p_sb[:], s_ps[:], AF.Exp,
                    scale=float(SCALE),
                    accum_out=partials[:, g:g + 1],
                )
                if pend is not None:
                    _transpose_pv(nc, ptps, ptsbp, identb, v,
                                  o_ps, *pend, n_vt)
                pend = (p_sb, g)
            _transpose_pv(nc, ptps, ptsbp, identb, v, o_ps, *pend, n_vt)

            denom = statp.tile([128, 1], F32, tag="denom")
            nc.vector.reduce_sum(
                denom[:], partials[:], axis=mybir.AxisListType.X
            )
            recip = statp.tile([128, 1], F32, tag="recip")
            nc.vector.reciprocal(recip[:], denom[:])
            nc.vector.tensor_scalar_mul(
                o_all[:, ntile * C:(ntile + 1) * C], o_ps[:], recip[:]
            )

        sT_ctx.close()

        # ---------------- projection with the swapaxes/reshape fold --------
        with tc.tile_pool(name="fsb", bufs=3) as fsbp:
            out_v = out[:].rearrange("(t two) d -> two t d", two=2)
            for i in range(2):
                for dc in range(C // 512):
                    for tt in range(CT):
                        ps = p512.tile([128, 512], F32, tag="g512")
                        for ct in range(CT):
                            nc.tensor.matmul(
                                ps[:],
                                lhsT=o_all[:, (i * CT + ct) * C + tt * 128: (i * CT + ct) * C + (tt + 1) * 128],
                                rhs=pw[:, dc * 4096 + ct * 512: dc * 4096 + (ct + 1) * 512],
                                start=(ct == 0),
                                stop=(ct == CT - 1),
                            )
                        f_sb = fsbp.tile([128, 512], F32, tag="fsb")
                        nc.vector.tensor_add(
                            f_sb[:], ps[:], bias[:, dc * 512:(dc + 1) * 512]
                        )
                        nc.sync.dma_start(
                            out_v[i, tt * 128:(tt + 1) * 128, dc * 512:(dc + 1) * 512],
                            f_sb[:],
                        )


def _transpose_pv(nc, ptps, ptsbp, identb, v, o_ps, p_sb, g, n_vt):
    # transpose the masked-in 128-tiles of p group g, then accumulate p@v
    mts = [mt for mt in range(g * (MS // 128), (g + 1) * (MS // 128))
           if mt < n_vt]
    if not mts:
        return
    pt_ps = ptps.tile([128, 128 * len(mts)], BF16, tag="ptps")
    for idx, mt in enumerate(mts):
        j = mt - g * (MS // 128)
        nc.tensor.transpose(
            pt_ps[:, idx * 128:(idx + 1) * 128],
            p_sb[:, j * 128:(j + 1) * 128],
            identb[:],
        )
    pt_sb = ptsbp.tile([128, 128 * len(mts)], BF16, tag="ptsb")
    nc.vector.tensor_copy(pt_sb[:], pt_ps[:])
    for idx, mt in enumerate(mts):
        for dc in range(C // 512):
            nc.tensor.matmul(
                o_ps[:, dc * 512:(dc + 1) * 512],
                lhsT=pt_sb[:, idx * 128:(idx + 1) * 128],
                rhs=v[:, mt * C + dc * 512: mt * C + (dc + 1) * 512],
                start=(mt == 0),
                stop=(mt == n_vt - 1),
            )


def _prep_w_lhs(w):
    # lhsT weights (y = x @ w^T form): dt-major [p, dt*1024 + ct*128 + dd]
    wt = w.T.reshape(CT, 128, CT, 128)          # [ct, p, dt, dd]
    return np.ascontiguousarray(
        wt.transpose(1, 2, 0, 3).reshape(128, CT * C)
    )


def _prep_w_rhs(w):
    # rhs weights (wv, pw): dc-major [p, dc*4096 + ct*512 + dd]
    wt = w.T.reshape(CT, 128, C // 512, 512)    # [ct, p, dc, dd]
    return np.ascontiguousarray(
        wt.transpose(1, 2, 0, 3).reshape(128, CT * C)
    )


def _prep_act(a):
    # a [rows, C] -> a.T [C, rows] grouped as [p, ct, rows]
    n = a.shape[0]
    return np.ascontiguousarray(a.T.reshape(CT, 128, n).transpose(1, 0, 2))


def prep_in_maps(x, support, attn_mask, qkv_w, proj_w, proj_b):
    x = np.asarray(x, dtype=np.float32)
    support = np.asarray(support, dtype=np.float32)
    attn_mask = np.asarray(attn_mask)
    qkv_w = np.asarray(qkv_w, dtype=np.float32)
    proj_w = np.asarray(proj_w, dtype=np.float32)
    proj_b = np.asarray(proj_b, dtype=np.float32)

    # host permutation: masked-in support rows first (stable order)
    mk = attn_mask.astype(bool)
    order = np.argsort(~mk, kind="stable")
    n_in = int(mk.sum())
    n_vt = max(1, -(-n_in // 128))
    maskp = mk[order].astype(np.float32)
    maskf = np.ascontiguousarray(maskp[:n_vt * 128].reshape(n_vt, 128).T)

    # fused q/k weights: s = x (wq^T wk) support^T, computed exactly on host
    wq = qkv_w[:C].astype(np.float64)
    wk = qkv_w[C:2 * C].astype(np.float64)
    W = (wq.T @ wk).astype(np.float32)
    wT = _prep_w_lhs(W.T)          # y = x @ (W^T)^T = x @ W
    wv = _prep_w_rhs(qkv_w[2 * C:])
    pw = _prep_w_rhs(proj_w).astype(ml_dtypes.bfloat16)
    biasb = np.ascontiguousarray(np.broadcast_to(proj_b, (128, C)))

    sperm = support[:, order, :]

    in_maps = []
    for b in range(B):
        in_maps.append({
            "xT": _prep_act(x[b]),
            "sT": _prep_act(sperm[b]),
            "wT": wT,
            "wvT": wv,
            "pwB": pw,
            "maskf": maskf,
            "biasb": biasb,
        })
    return in_maps, n_vt


def kernel(x, support, attn_mask, qkv_w, proj_w, proj_b):
    in_maps, n_vt = prep_in_maps(x, support, attn_mask, qkv_w, proj_w, proj_b)
    key = ("nc", n_vt)
    if key not in _CACHE:
        _CACHE[key] = _build_program(n_vt)
    nc = _CACHE[key]
    _CACHE["nc"] = nc

    res = run_bass_kernel_spmd(nc, in_maps, core_ids=list(range(B)))
    return np.stack([res.results[b]["out"] for b in range(B)], axis=0)


# revision 41
# speedup vs baseline: 1.5847x; 1.0166x over previous
"""Trainium2 Bass kernel for CrossAttention (B=8, N=M=2048, C=1024), fp32.

Sharding: data-parallel — one batch element per NeuronCore (8 cores).

Key algebraic fold: the module never reshapes into heads, so
  s = (x wq^T)(support wk^T)^T = x (wq^T wk) support^T.
W = wq^T wk is precomputed exactly on the host, which deletes the entire
k-projection GEMM (and its 8MB kT buffer) from the device: the kernel
computes xw = x @ W once, and the s-matmul contracts xw directly against
support^T — which is already the DMA input layout.

Per-core computation (batch b), with a host-side permutation of the support
rows that places the attn_mask==1 ("masked-in") rows first:
  xwT[c', n] = (x @ W)^T                    (all n up front; PE-bound phase)
  v[m, d]  = (support_perm @ wv^T) * mask   (ONLY the first n_vt tiles of 128
                                             rows; masked-out rows contribute
                                             nothing to p@v so they are never
                                             built and never contracted)
  p = exp(SCALE * xw support^T)  (no max subtraction: logits ~ N(0, 8);
                                  per-row sums give the softmax denominator,
                                  which spans ALL support positions)
  o[n, d] = (p[:, :n_vt*128] @ v) / rowsum_all(p)
  The reference's  out.swapaxes(1,2).reshape(N, C)  permutation satisfies
  o_perm[2t+i, c] = o[1024*i + c, t], so the final projection becomes
  final[2t+i, d'] = sum_c o[1024*i + c, t] * proj_w[d', c]  — a plain matmul
  with o-block-i rows as the contraction dim, written out with row stride 2.

fp32r operands (full-rate PE streaming) for everything feeding the softmax
logits; bf16 for the p/v/o/proj operands where quantization error (~0.4%)
is far inside the 2e-2 gate — bf16 drops the PE transpose cost to 1.0
cyc/row and lets v + o + pw all stay SBUF-resident (no DRAM bounce).

Scheduling notes:
 - Tile-pool alloc boundaries serialize against the prior phase's readers
   of the reused space, so a fresh pool's DMAs can't prefetch.  PSUM pools
   and the big resident tiles (sT on the right allocator side, xw, v, o)
   are opened once; only W/xq (phase XW) and wv (phase V) are transient.
 - The attention inner loop is software-pipelined (s of group g+1 issues
   before transpose/pv of group g) so the PE never waits on exp (Act) or
   the psum->sbuf copies (DVE).
 - An idle PE resets the 0.65/1.2/2.4 GHz p-state ramp (~+1us per stall).
   A warm-up matmul chain covers the initial DMA wait, and filler matmuls
   pad the known DMA-gated windows (W stream at start, wv at phase V).
 - DMA transfers are batched (>=0.5MB strided slabs): per-transfer
   overhead is ~0.4us.
"""

import sys

sys.path.insert(0, "/opt/trn_rl_repo")

import numpy as np
import ml_dtypes

import concourse.bass as bass
import concourse.tile as tile
from concourse import bacc, mybir
from concourse.bass_utils import run_bass_kernel_spmd
from concourse.masks import make_identity

F32 = mybir.dt.float32
F32R = mybir.dt.float32r
BF16 = mybir.dt.bfloat16
AF = mybir.ActivationFunctionType

B, N, M, C = 8, 2048, 2048, 1024
CT = C // 128          # 8 c-tiles (contraction / channel partition tiles)
MT = M // 128          # 16 m-tiles
SCALE = (C // 8) ** -0.5
QS = 256               # x load / xw psum sub-chunk
MS = 512               # attention m-group (s matmul free dim)
WARMUP_MM = 36         # PE p-state warm-up matmuls during startup DMA

_CACHE = {}


def _build_program(n_vt):
    nc = bacc.Bacc(
        "TRN2",
        target_bir_lowering=False,
        debug=False,
        enable_asserts=False,
        num_devices=8,
    )

    xT = nc.dram_tensor("xT", [128, CT, N], F32, kind="ExternalInput")
    sT = nc.dram_tensor("sT", [128, CT, M], F32, kind="ExternalInput")
    wT = nc.dram_tensor("wT", [128, CT * C], F32, kind="ExternalInput")
    wvT = nc.dram_tensor("wvT", [128, CT * C], F32, kind="ExternalInput")
    pwB = nc.dram_tensor("pwB", [128, CT * C], BF16, kind="ExternalInput")
    maskf = nc.dram_tensor("maskf", [128, n_vt], F32, kind="ExternalInput")
    biasb = nc.dram_tensor("biasb", [128, C], F32, kind="ExternalInput")
    out = nc.dram_tensor("out", [N, C], F32, kind="ExternalOutput")

    with tile.TileContext(nc, pool_alloc_mode="queue") as tc:
        _trace_kernel(tc, n_vt, xT, sT, wT, wvT, pwB, maskf, biasb, out)
    nc.compile()
    return nc


def _dma_act(nc, atile, adram, col0, cols, split=1):
    # batched strided load (per-transfer overhead ~0.4us); split>1 only for
    # the very first load where startup latency matters
    step = CT // split
    for i in range(split):
        nc.sync.dma_start(
            atile[:, i * step:(i + 1) * step, :],
            adram[:, i * step:(i + 1) * step, col0:col0 + cols].bitcast(F32R),
        )


def _trace_kernel(tc, n_vt, xT, sT, wT, wvT, pwB, maskf, biasb, out):
    nc = tc.nc

    from contextlib import ExitStack

    with ExitStack() as ctx:
        persist = ctx.enter_context(tc.tile_pool(name="persist", bufs=1))

        # program-lifetime PSUM pools, shared across phases by tag (slots
        # are bank-granular; fresh per-phase pools add boundary stalls)
        p512 = ctx.enter_context(tc.tile_pool(name="p512", bufs=2,
                                              space="PSUM"))
        p256 = ctx.enter_context(tc.tile_pool(name="p256", bufs=2,
                                              space="PSUM"))
        ptps = ctx.enter_context(tc.tile_pool(name="ptp", bufs=2,
                                              space="PSUM"))
        ops = ctx.enter_context(tc.tile_pool(name="pop", bufs=1,
                                             space="PSUM"))

        # p-state warm-up: memset a scratch tile (no input deps), then a
        # chained matmul run keeps the PE busy through the initial DMA wait
        wub = persist.tile([128, 256], F32, tag="wub")
        nc.vector.memset(wub[:], 1.0)

        def filler(n):
            # f32r is a PE-side view: memset must target a plain f32 tile
            # (f32r memset fails the neuronxcc ISA check), bitcast at use
            for _ in range(n):
                ps = p256.tile([128, 256], F32, tag="g256")
                nc.tensor.matmul(ps[:], lhsT=wub[:, 0:128].bitcast(F32R),
                                 rhs=wub[:].bitcast(F32R),
                                 start=True, stop=True)

        filler(WARMUP_MM)

        ident0 = persist.tile([128, 128], F32, tag="ident0")
        make_identity(nc, ident0[:])
        identb = persist.tile([128, 128], BF16, tag="identb")
        nc.scalar.copy(identb[:], ident0[:])
        maskt = persist.tile([128, n_vt], F32, tag="maskt")
        nc.sync.dma_start(maskt[:], maskf[:])
        # first two ct-slices of wv dc0, loaded inside phase XW's window so
        # the V phase starts without waiting on the wv pool boundary
        wvA = persist.tile([128, 1024], F32R, tag="wvA")

        # ---- long-lived operands -----------------------------------------
        # support^T resident on the right allocator side: it is both the
        # s-matmul rhs and the v-build lhsT, and releases before proj
        sT_ctx = ExitStack()
        sTp = sT_ctx.enter_context(tc.tile_pool(name="sTp", bufs=1,
                                                side="right"))
        sTr = sTp.tile([128, CT, M], F32R, tag="sTr")     # 8 MB
        vp = ctx.enter_context(tc.tile_pool(name="vp", bufs=1))
        v = vp.tile([128, n_vt * C], BF16, tag="v")       # 2 MB @ n_vt=8
        xwp = ctx.enter_context(tc.tile_pool(name="xwp", bufs=1))
        xw = xwp.tile([128, CT * N], F32R, tag="xw")      # 8 MB

        # ---------------- phase XW: (x @ W)^T for all n --------------------
        # W is dt-major: the first group needs only cols [0:1024]
        cw = (CT * C) // 8
        with (
            tc.tile_pool(name="wp", bufs=1) as wp,
            tc.tile_pool(name="xqp", bufs=2) as xqp,
        ):
            w = wp.tile([128, CT * C], F32R, tag="w")
            for sub in range(N // QS):
                xq = xqp.tile([128, CT, QS], F32R, tag="xq")
                first = sub == 0
                if first:
                    nc.sync.dma_start(
                        w[:, 0:cw], wT[:, 0:cw].bitcast(F32R))
                    _dma_act(nc, xq, xT, 0, QS, split=2)
                    for i in range(1, CT):
                        nc.sync.dma_start(
                            w[:, i * cw:(i + 1) * cw],
                            wT[:, i * cw:(i + 1) * cw].bitcast(F32R),
                        )
                else:
                    _dma_act(nc, xq, xT, sub * QS, QS)
                # W streams at ~1.8us/chunk vs 0.85us/group burn in the
                # first sub: pad the supply gaps to avoid p-state resets
                for dt in range(CT):
                    ps = p256.tile([128, QS], F32, tag="g256")
                    for ct in range(CT):
                        nc.tensor.matmul(
                            ps[:],
                            lhsT=w[:, dt * C + ct * 128: dt * C + (ct + 1) * 128],
                            rhs=xq[:, ct, :],
                            start=(ct == 0),
                            stop=(ct == CT - 1),
                        )
                    nc.scalar.copy(
                        xw[:, dt * N + sub * QS: dt * N + (sub + 1) * QS],
                        ps[:],
                    )
                    if first:
                        filler(3)
            # support stream into the already-open right-side pool: the
            # first half (v-build + attention groups 0/1) leads the queue
            for i in range(2):
                nc.sync.dma_start(
                    sTr[:, :, i * 512:(i + 1) * 512],
                    sT[:, :, i * 512:(i + 1) * 512].bitcast(F32R),
                )
            nc.sync.dma_start(wvA[:], wvT[:, 0:1024].bitcast(F32R))

        # ---------------- phase V: v[m, d] for the masked-in tiles ---------
        # pw/bias land behind wv and the sT tail in the DMA queue; wv's
        # space frees into W/xq's (boundary on the last xw-build read)
        pwp = ctx.enter_context(tc.tile_pool(name="pwp", bufs=1))
        pw = pwp.tile([128, CT * C], BF16, tag="pw")
        bias = pwp.tile([128, C], F32, tag="bias")
        with tc.tile_pool(name="wvp", bufs=1) as wvp:
            # remaining wv: dc0's tail at per-ct granularity (the first
            # v-groups trickle-feed right at the phase boundary), dc1 bulk
            # full-size tile (tail unused) so the freed hole still fits o_all
            wv = wvp.tile([128, CT * C], F32R, tag="wv")
            for i in range(6):
                nc.sync.dma_start(
                    wv[:, i * 512:(i + 1) * 512],
                    wvT[:, 1024 + i * 512: 1024 + (i + 1) * 512].bitcast(F32R),
                )
            for i in range(2):
                nc.sync.dma_start(
                    wv[:, 3072 + i * 2048: 3072 + (i + 1) * 2048],
                    wvT[:, 4096 + i * 2048: 4096 + (i + 1) * 2048].bitcast(F32R),
                )
            for i in range(2, 4):
                nc.sync.dma_start(
                    sTr[:, :, i * 512:(i + 1) * 512],
                    sT[:, :, i * 512:(i + 1) * 512].bitcast(F32R),
                )
            for i in range(4):
                nc.sync.dma_start(
                    pw[:, i * 2048:(i + 1) * 2048],
                    pwB[:, i * 2048:(i + 1) * 2048],
                )
            nc.sync.dma_start(bias[:], biasb[:])

            filler(4)
            for dc in range(C // 512):
                for mt in range(n_vt):
                    ps = p512.tile([128, 512], F32, tag="g512")
                    for ct in range(CT):
                        if dc == 0 and ct < 2:
                            rhs = wvA[:, ct * 512:(ct + 1) * 512]
                        else:
                            off = dc * 4096 + ct * 512 - 1024
                            rhs = wv[:, off:off + 512]
                        nc.tensor.matmul(
                            ps[:],
                            lhsT=sTr[:, ct, mt * 128:(mt + 1) * 128],
                            rhs=rhs,
                            start=(ct == 0),
                            stop=(ct == CT - 1),
                        )
                    nc.vector.tensor_scalar_mul(
                        v[:, mt * C + dc * 512: mt * C + (dc + 1) * 512],
                        ps[:],
                        maskt[:, mt:mt + 1],
                    )

        # ---------------- attention: pipelined s / exp / T / pv ------------
        psbp = ctx.enter_context(tc.tile_pool(name="psb", bufs=2))
        ptsbp = ctx.enter_context(tc.tile_pool(name="ptsb", bufs=2))
        statp = ctx.enter_context(tc.tile_pool(name="stat", bufs=4))
        op = ctx.enter_context(tc.tile_pool(name="op", bufs=1))
        o_all = op.tile([128, N // 128 * C], BF16, tag="o_all")  # 4 MB

        NG = M // MS          # 4 m-groups per n-tile
        for ntile in range(N // 128):
            partials = statp.tile([128, NG], F32, tag="partials")
            o_ps = ops.tile([128, C], F32, tag="ops")

            # two-stage software pipeline: transpose+copy of group g-1
            # and pv of group g-2 issue under the s-matmuls of group g, so
            # the PE never waits on exp (Act) or the pt copy (DVE) — the
            # last pv's copy hides under the final s-group
            tcop = {}   # g -> pt_sb (copy in flight)
            for g in range(NG + 2):
                if g < NG:
                    s_ps = p512.tile([128, MS], F32, tag="g512")
                    for ct in range(CT):
                        nc.tensor.matmul(
                            s_ps[:],
                            lhsT=xw[:, ct * N + ntile * 128: ct * N + (ntile + 1) * 128],
                            rhs=sTr[:, ct, g * MS:(g + 1) * MS],
                            start=(ct == 0),
                            stop=(ct == CT - 1),
                        )
                    p_sb = psbp.tile([128, MS], BF16, tag="psb")
                    nc.scalar.activation(
                        p_sb[:], s_ps[:], AF.Exp,
                        scale=float(SCALE),
                        accum_out=partials[:, g:g + 1],
                    )
                    tcop[g] = p_sb
                if g - 1 in tcop:
                    tcop[g - 1] = _transpose_copy(
                        nc, ptps, ptsbp, identb, tcop[g - 1], g - 1, n_vt)
                if g - 2 in tcop:
                    _pv(nc, v, o_ps, tcop.pop(g - 2), g - 2, n_vt)

            denom = statp.tile([128, 1], F32, tag="denom")
            nc.vector.reduce_sum(
                denom[:], partials[:], axis=mybir.AxisListType.X
            )
            recip = statp.tile([128, 1], F32, tag="recip")
            nc.vector.reciprocal(recip[:], denom[:])
            nc.vector.tensor_scalar_mul(
                o_all[:, ntile * C:(ntile + 1) * C], o_ps[:], recip[:]
            )

        sT_ctx.close()

        # ---------------- projection with the swapaxes/reshape fold --------
        with tc.tile_pool(name="fsb", bufs=3) as fsbp:
            out_v = out[:].rearrange("(t two) d -> two t d", two=2)
            for i in range(2):
                for dc in range(C // 512):
                    for tt in range(CT):
                        ps = p512.tile([128, 512], F32, tag="g512")
                        for ct in range(CT):
                            nc.tensor.matmul(
                                ps[:],
                                lhsT=o_all[:, (i * CT + ct) * C + tt * 128: (i * CT + ct) * C + (tt + 1) * 128],
                                rhs=pw[:, dc * 4096 + ct * 512: dc * 4096 + (ct + 1) * 512],
                                start=(ct == 0),
                                stop=(ct == CT - 1),
                            )
                        f_sb = fsbp.tile([128, 512], F32, tag="fsb")
                        nc.vector.tensor_add(
                            f_sb[:], ps[:], bias[:, dc * 512:(dc + 1) * 512]
                        )
                        nc.sync.dma_start(
                            out_v[i, tt * 128:(tt + 1) * 128, dc * 512:(dc + 1) * 512],
                            f_sb[:],
                        )


def _mts_of(g, n_vt):
    return [mt for mt in range(g * (MS // 128), (g + 1) * (MS // 128))
            if mt < n_vt]


def _transpose_copy(nc, ptps, ptsbp, identb, p_sb, g, n_vt):
    # transpose the masked-in 128-tiles of p group g into SBUF (pv later)
    mts = _mts_of(g, n_vt)
    if not mts:
        return None
    pt_ps = ptps.tile([128, 128 * len(mts)], BF16, tag="ptps")
    for idx, mt in enumerate(mts):
        j = mt - g * (MS // 128)
        nc.tensor.transpose(
            pt_ps[:, idx * 128:(idx + 1) * 128],
            p_sb[:, j * 128:(j + 1) * 128],
            identb[:],
        )
    pt_sb = ptsbp.tile([128, 128 * len(mts)], BF16, tag="ptsb")
    nc.vector.tensor_copy(pt_sb[:], pt_ps[:])
    return pt_sb


def _pv(nc, v, o_ps, pt_sb, g, n_vt):
    mts = _mts_of(g, n_vt)
    if not mts:
        return
    for idx, mt in enumerate(mts):
        for dc in range(C // 512):
            nc.tensor.matmul(
                o_ps[:, dc * 512:(dc + 1) * 512],
                lhsT=pt_sb[:, idx * 128:(idx + 1) * 128],
                rhs=v[:, mt * C + dc * 512: mt * C + (dc + 1) * 512],
                start=(mt == 0),
                stop=(mt == n_vt - 1),
            )


def _prep_w_lhs(w):
    # lhsT weights (y = x @ w^T form): dt-major [p, dt*1024 + ct*128 + dd]
    wt = w.T.reshape(CT, 128, CT, 128)          # [ct, p, dt, dd]
    return np.ascontiguousarray(
        wt.transpose(1, 2, 0, 3).reshape(128, CT * C)
    )


def _prep_w_rhs(w):
    # rhs weights (wv, pw): dc-major [p, dc*4096 + ct*512 + dd]
    wt = w.T.reshape(CT, 128, C // 512, 512)    # [ct, p, dc, dd]
    return np.ascontiguousarray(
        wt.transpose(1, 2, 0, 3).reshape(128, CT * C)
    )


def _prep_act(a):
    # a [rows, C] -> a.T [C, rows] grouped as [p, ct, rows]
    n = a.shape[0]
    return np.ascontiguousarray(a.T.reshape(CT, 128, n).transpose(1, 0, 2))


def prep_in_maps(x, support, attn_mask, qkv_w, proj_w, proj_b):
    x = np.asarray(x, dtype=np.float32)
    support = np.asarray(support, dtype=np.float32)
    attn_mask = np.asarray(attn_mask)
    qkv_w = np.asarray(qkv_w, dtype=np.float32)
    proj_w = np.asarray(proj_w, dtype=np.float32)
    proj_b = np.asarray(proj_b, dtype=np.float32)

    # host permutation: masked-in support rows first (stable order)
    mk = attn_mask.astype(bool)
    order = np.argsort(~mk, kind="stable")
    n_in = int(mk.sum())
    n_vt = max(1, -(-n_in // 128))
    maskp = mk[order].astype(np.float32)
    maskf = np.ascontiguousarray(maskp[:n_vt * 128].reshape(n_vt, 128).T)

    # fused q/k weights: s = x (wq^T wk) support^T, computed exactly on host
    wq = qkv_w[:C].astype(np.float64)
    wk = qkv_w[C:2 * C].astype(np.float64)
    W = (wq.T @ wk).astype(np.float32)
    wT = _prep_w_lhs(W.T)          # y = x @ (W^T)^T = x @ W
    wv = _prep_w_rhs(qkv_w[2 * C:])
    pw = _prep_w_rhs(proj_w).astype(ml_dtypes.bfloat16)
    biasb = np.ascontiguousarray(np.broadcast_to(proj_b, (128, C)))

    sperm = support[:, order, :]

    in_maps = []
    for b in range(B):
        in_maps.append({
            "xT": _prep_act(x[b]),
            "sT": _prep_act(sperm[b]),
            "wT": wT,
            "wvT": wv,
            "pwB": pw,
            "maskf": maskf,
            "biasb": biasb,
        })
    return in_maps, n_vt


def kernel(x, support, attn_mask, qkv_w, proj_w, proj_b):
    in_maps, n_vt = prep_in_maps(x, support, attn_mask, qkv_w, proj_w, proj_b)
    key = ("nc", n_vt)
    if key not in _CACHE:
        _CACHE[key] = _build_program(n_vt)
    nc = _CACHE[key]
    _CACHE["nc"] = nc

    res = run_bass_kernel_spmd(nc, in_maps, core_ids=list(range(B)))
    return np.stack([res.results[b]["out"] for b in range(B)], axis=0)
